# revision 25
# baseline (speedup 1.0000x reference)
"""Trainium2 Bass kernel for nn_AttentionMemory (sparse_attention).

Reference computation (per batch b):
    mk = Mk[b].reshape(CK, N); qk = Qk[b].reshape(CK, N)
    affinity[m, q] = softmax_m( (2*mk[:,m]@qk[:,q] - |mk[:,m]|^2) / sqrt(CK) )

Sharding: 8 cores = 4 batches x 2 query-halves. Each core computes the full
memory (softmax) axis for 2048 of one batch's queries — no collectives.

Per-core layout: queries on partitions (16 q-tiles of 128), memory positions
on the free axis. Inputs are pre-cast to fp16 on the host (halves input DMA
and doubles PE streaming rate vs fp32r; rel-err budget 2e-2 >> fp16's ~2e-3).
Per q-tile the 4096-wide pre-softmax row is built in two [128, 2048] PSUM
tiles (4 banks each, ping-pong):
    ps[:, c*512:+512] = matmul(-0.5*ones, mk*mk) accum matmul(qk_tile, mk)
ScalarE applies one exp per PSUM tile (2048 wide, amortizes the ~352-cycle
per-instruction overhead; bf16 output). DVE computes the softmax denominator
with a tensor_scalar pass (bf16 4x mode) whose accum_out is the row sum —
cheaper than ScalarE's accum_out, which costs a separate ~360 ns
ACTIVATION_READ_ACCUMULATOR per activation — then a reciprocal and one
tensor_scalar multiply produce the normalized bf16 output; one 1 MiB DMA per
q-tile writes [128, 4096]. The logits are bounded (~[-30, +8]) so no
max-subtraction pass is needed. The host casts bf16->fp32 and transposes
[q, m] -> [m, q] while gathering.

Walrus caps instructions at one sync wait; _strip_self_waits spills extra
waits onto single-wait Drain instructions (semantically equivalent — waits
are an AND over monotonic semaphores, executed in order on one sequencer).
"""
import math
import numpy as np

import bass_rust
from concourse import bass, tile, mybir
from concourse.bass_utils import run_bass_kernel_spmd

B, CK, HH, WW = 4, 128, 64, 64
N = HH * WW            # 4096 memory positions / queries per batch
QH = N // 2            # 2048 queries per core
N_CORES = 8
QTILE = 128            # queries per q-tile (PSUM partition dim)
MCHUNK = 512           # one PSUM bank of fp32
MWIDE = 1024           # memory cols per matmul (fp16 moving operand max)
MHALF = 2048           # memory cols per PSUM tile / exp instruction
SCALE = 2.0 / math.sqrt(CK)
F32 = mybir.dt.float32
F16 = mybir.dt.float16
BF16 = mybir.dt.bfloat16


def _build():
    nc = bass.Bass("TRN2", target_bir_lowering=False, debug=False,
                   num_devices=N_CORES)
    mk_d = nc.dram_tensor("mk", [CK, N], F16, kind="ExternalInput").ap()
    qk_d = nc.dram_tensor("qk", [CK, QH], F16, kind="ExternalInput").ap()
    out_d = nc.dram_tensor("out", [QH, N], BF16, kind="ExternalOutput").ap()

    n_qt = QH // QTILE          # 16
    with tile.TileContext(nc) as tc:
        with tc.tile_pool(name="inp", bufs=1) as inp_pool, \
             tc.tile_pool(name="exp", bufs=2) as exp_pool, \
             tc.tile_pool(name="outb", bufs=3) as out_pool, \
             tc.tile_pool(name="small", bufs=4) as small_pool, \
             tc.tile_pool(name="psum", bufs=2, space="PSUM") as psum_pool:

            mk_sb = inp_pool.tile([CK, N], F16, tag="mk")
            qk_sb = inp_pool.tile([CK, QH], F16, tag="qk")
            mksq = inp_pool.tile([CK, N], F16, tag="mksq")
            mhalf = inp_pool.tile([128, QTILE], F16, tag="mhalf")
            nc.vector.memset(mhalf[:], -0.5)
            # Split input DMAs so the first q-tile's operands land early:
            # the mk quarters gate mksq -> first matmuls, so they go first;
            # qk's first stationary tile (tiny) rides between them and the
            # bulk of qk loads last. A tiny primer transfer absorbs the
            # one-time DMA ring/HBM-path init ahead of the critical load.
            primer = inp_pool.tile([CK, 2], F16, tag="primer")
            nc.sync.dma_start(out=primer[:], in_=mk_d[:, 0:2])
            nc.sync.dma_start(out=mk_sb[:, 0:1024], in_=mk_d[:, 0:1024])
            nc.sync.dma_start(out=mk_sb[:, 1024:2048], in_=mk_d[:, 1024:2048])
            nc.sync.dma_start(out=qk_sb[:, 0:QTILE], in_=qk_d[:, 0:QTILE])
            nc.sync.dma_start(out=mk_sb[:, 2048:3072], in_=mk_d[:, 2048:3072])
            nc.sync.dma_start(out=mk_sb[:, 3072:4096], in_=mk_d[:, 3072:4096])
            nc.sync.dma_start(out=qk_sb[:, QTILE:QH], in_=qk_d[:, QTILE:QH])
            # First 512 split out so the very first matmul can start early.
            nc.vector.tensor_mul(mksq[:, 0:512], mk_sb[:, 0:512],
                                 mk_sb[:, 0:512])
            nc.vector.tensor_mul(mksq[:, 512:1024], mk_sb[:, 512:1024],
                                 mk_sb[:, 512:1024])
            for c in range(1, 4):
                nc.vector.tensor_mul(mksq[:, c * 1024:(c + 1) * 1024],
                                     mk_sb[:, c * 1024:(c + 1) * 1024],
                                     mk_sb[:, c * 1024:(c + 1) * 1024])

            # Warm the PE's HAM clock gate (K=4/8 -> 8/8 needs ~3.4us of
            # sustained activity) with throwaway matmuls that only need the
            # memset mhalf tile, overlapping the input DMA wait. The first
            # real matmuls then run at 2.4 GHz instead of 1.2.
            warm_ps = psum_pool.tile([QTILE, MHALF], F32, tag="ps")
            for w in range(44):
                nc.tensor.matmul(warm_ps[:, 0:QTILE], mhalf[:], mhalf[:],
                                 start=True, stop=True)

            for t in range(n_qt):
                qk_t = qk_sb[:, t * QTILE:(t + 1) * QTILE]
                exp_t = exp_pool.tile([QTILE, N], BF16, tag="exp")
                parts = small_pool.tile([QTILE, 2], F32, tag="parts")
                s_t = small_pool.tile([QTILE, 1], F32, tag="S")
                rec_t = small_pool.tile([QTILE, 1], F32, tag="rec")
                o = out_pool.tile([QTILE, N], BF16, tag="o")
                for h in range(2):
                    ps = psum_pool.tile([QTILE, MHALF], F32, tag="ps")
                    for c in range(4):
                        m0 = h * MHALF + c * MCHUNK
                        nc.tensor.matmul(ps[:, c * MCHUNK:(c + 1) * MCHUNK],
                                         mhalf[:], mksq[:, m0:m0 + MCHUNK],
                                         start=True, stop=False)
                    for c in range(4):
                        m0 = h * MHALF + c * MCHUNK
                        nc.tensor.matmul(ps[:, c * MCHUNK:(c + 1) * MCHUNK],
                                         qk_t, mk_sb[:, m0:m0 + MCHUNK],
                                         start=False, stop=True)
                    nc.scalar.activation(
                        exp_t[:, h * MHALF:(h + 1) * MHALF], ps[:],
                        mybir.ActivationFunctionType.Exp, scale=SCALE,
                        accum_out=parts[:, h:h + 1])
                # Denominator (partials from the 2 activations) + normalize.
                nc.vector.tensor_add(s_t[:], parts[:, 0:1], parts[:, 1:2])
                nc.vector.reciprocal(rec_t[:], s_t[:])
                # Normalize + store per half: output DMA of half h starts
                # while half h+1 is still being scaled (shorter tail).
                for h in range(2):
                    nc.vector.tensor_scalar_mul(
                        o[:, h * MHALF:(h + 1) * MHALF],
                        exp_t[:, h * MHALF:(h + 1) * MHALF], rec_t[:])
                    nc.sync.dma_start(
                        out=out_d[t * QTILE:(t + 1) * QTILE,
                                  h * MHALF:(h + 1) * MHALF],
                        in_=o[:, h * MHALF:(h + 1) * MHALF])
    _strip_self_waits(nc)
    return nc


def _strip_self_waits(nc):
    """Walrus rejects instructions carrying more than one sync wait.

    Conservative fix: for any instruction with N>1 waits, keep the last wait
    on the instruction and spill the other N-1 onto single-wait Drain
    instructions inserted immediately before it on the same engine. All waits
    still execute, in program order, on the same sequencer; semaphores are
    monotonic so splitting an AND of waits into a sequence is equivalent.
    """
    for fn in nc.m.functions:
        for blk in fn.blocks:
            il = blk.instructions
            new_il = []
            changed = False
            for ins in il:
                si = getattr(ins, "sync_info", None)
                if si is not None and len(si.on_wait) > 1:
                    changed = True
                    waits = list(si.on_wait)
                    for k, w in enumerate(waits[:-1]):
                        d = mybir.InstDrain(
                            name=f"{ins.name}_w{k}",
                            ins=[], outs=[], bass_is_fusable=False)
                        d.engine = ins.engine
                        d.sync_info = bass_rust.SyncInfo(on_wait=[w],
                                                         on_update=[])
                        new_il.append(d)
                    ins.sync_info = bass_rust.SyncInfo(on_wait=[waits[-1]],
                                                      on_update=si.on_update)
                new_il.append(ins)
            if changed:
                blk.instructions = new_il


_NC_CACHE = None


def _make_in_maps(Mk: np.ndarray, Qk: np.ndarray) -> list[dict]:
    Mk = np.ascontiguousarray(np.asarray(Mk), dtype=np.float32)
    Qk = np.ascontiguousarray(np.asarray(Qk), dtype=np.float32)
    in_maps = []
    for c in range(N_CORES):
        b, half = c // 2, c % 2
        mk = np.ascontiguousarray(Mk[b].reshape(CK, N).astype(np.float16))
        qk = np.ascontiguousarray(
            Qk[b].reshape(CK, N)[:, half * QH:(half + 1) * QH]
            .astype(np.float16))
        in_maps.append({"mk": mk, "qk": qk})
    return in_maps


def kernel(Mk: np.ndarray, Qk: np.ndarray) -> np.ndarray:
    global _NC_CACHE
    if _NC_CACHE is None:
        _NC_CACHE = _build()
    nc = _NC_CACHE

    in_maps = _make_in_maps(Mk, Qk)

    res = run_bass_kernel_spmd(nc, in_maps, core_ids=list(range(N_CORES)))

    out = np.empty((B, N, N), dtype=np.float32)
    for c in range(N_CORES):
        b, half = c // 2, c % 2
        out[b, :, half * QH:(half + 1) * QH] = \
            res.results[c]["out"].astype(np.float32).T
    return out


# revision 27
# speedup vs baseline: 1.1729x; 1.1729x over previous
"""Trainium2 Bass kernel for nn_AttentionMemory (sparse_attention).

Reference computation (per batch b):
    mk = Mk[b].reshape(CK, N); qk = Qk[b].reshape(CK, N)
    affinity[m, q] = softmax_m( (2*mk[:,m]@qk[:,q] - |mk[:,m]|^2) / sqrt(CK) )

Sharding: 8 cores = 4 batches x 2 query-halves. Each core computes the full
memory (softmax) axis for 2048 of one batch's queries — no collectives.

Per-core layout: queries on partitions (16 q-tiles of 128), memory positions
on the free axis. Inputs are pre-cast to fp16 on the host (halves input DMA
and doubles PE streaming rate vs fp32r; rel-err budget 2e-2 >> fp16's ~2e-3).
Per q-tile the 4096-wide pre-softmax row is built in two [128, 2048] PSUM
tiles (4 banks each, ping-pong):
    ps[:, c*512:+512] = matmul(-0.5*ones, mk*mk) accum matmul(qk_tile, mk)
ScalarE applies one exp per PSUM tile (2048 wide, amortizes the ~352-cycle
per-instruction overhead; bf16 output). DVE computes the softmax denominator
with a tensor_scalar pass (bf16 4x mode) whose accum_out is the row sum —
cheaper than ScalarE's accum_out, which costs a separate ~360 ns
ACTIVATION_READ_ACCUMULATOR per activation — then a reciprocal and one
tensor_scalar multiply produce the normalized bf16 output; one 1 MiB DMA per
q-tile writes [128, 4096]. The logits are bounded (~[-30, +8]) so no
max-subtraction pass is needed. The host casts bf16->fp32 and transposes
[q, m] -> [m, q] while gathering.

Walrus caps instructions at one sync wait; _strip_self_waits spills extra
waits onto single-wait Drain instructions (semantically equivalent — waits
are an AND over monotonic semaphores, executed in order on one sequencer).
"""
import math
import numpy as np

import bass_rust
from concourse import bass, tile, mybir
from concourse.bass_utils import run_bass_kernel_spmd

B, CK, HH, WW = 4, 128, 64, 64
N = HH * WW            # 4096 memory positions / queries per batch
QH = N // 2            # 2048 queries per core
N_CORES = 8
QTILE = 128            # queries per q-tile (PSUM partition dim)
MCHUNK = 512           # one PSUM bank of fp32
MWIDE = 1024           # memory cols per matmul (fp16 moving operand max)
MHALF = 2048           # memory cols per PSUM tile / exp instruction
SCALE = 2.0 / math.sqrt(CK)
F32 = mybir.dt.float32
F16 = mybir.dt.float16
BF16 = mybir.dt.bfloat16


def _build():
    nc = bass.Bass("TRN2", target_bir_lowering=False, debug=False,
                   num_devices=N_CORES)
    mk_d = nc.dram_tensor("mk", [CK, N], F16, kind="ExternalInput").ap()
    qk_d = nc.dram_tensor("qk", [CK, QH], F16, kind="ExternalInput").ap()
    out_d = nc.dram_tensor("out", [QH, N], BF16, kind="ExternalOutput").ap()

    n_qt = QH // QTILE          # 16
    with tile.TileContext(nc) as tc:
        with tc.tile_pool(name="inp", bufs=1) as inp_pool, \
             tc.tile_pool(name="exp", bufs=2) as exp_pool, \
             tc.tile_pool(name="outb", bufs=3) as out_pool, \
             tc.tile_pool(name="small", bufs=4) as small_pool, \
             tc.tile_pool(name="psum", bufs=2, space="PSUM") as psum_pool:

            mk_sb = inp_pool.tile([CK, N], F16, tag="mk")
            qk_sb = inp_pool.tile([CK, QH], F16, tag="qk")
            mksq = inp_pool.tile([CK, N], F16, tag="mksq")
            mhalf = inp_pool.tile([128, QTILE], F16, tag="mhalf")
            nc.vector.memset(mhalf[:], -0.5)
            # Split input DMAs so the first q-tile's operands land early:
            # the mk quarters gate mksq -> first matmuls, so they go first;
            # qk's first stationary tile (tiny) rides between them and the
            # bulk of qk loads last.
            nc.sync.dma_start(out=mk_sb[:, 0:1024], in_=mk_d[:, 0:1024])
            nc.sync.dma_start(out=mk_sb[:, 1024:2048], in_=mk_d[:, 1024:2048])
            nc.sync.dma_start(out=qk_sb[:, 0:QTILE], in_=qk_d[:, 0:QTILE])
            nc.sync.dma_start(out=mk_sb[:, 2048:3072], in_=mk_d[:, 2048:3072])
            nc.sync.dma_start(out=mk_sb[:, 3072:4096], in_=mk_d[:, 3072:4096])
            nc.sync.dma_start(out=qk_sb[:, QTILE:QH], in_=qk_d[:, QTILE:QH])
            for c in range(4):
                nc.vector.tensor_mul(mksq[:, c * 1024:(c + 1) * 1024],
                                     mk_sb[:, c * 1024:(c + 1) * 1024],
                                     mk_sb[:, c * 1024:(c + 1) * 1024])

            # Warm the PE's HAM clock gate (K=4/8 -> 8/8 needs ~3.4us of
            # sustained activity) with throwaway matmuls that only need the
            # memset mhalf tile, overlapping the input DMA wait. The first
            # real matmuls then run at 2.4 GHz instead of 1.2.
            warm_ps = psum_pool.tile([QTILE, MHALF], F32, tag="ps")
            for w in range(44):
                nc.tensor.matmul(warm_ps[:, 0:QTILE], mhalf[:], mhalf[:],
                                 start=True, stop=True)

            for t in range(n_qt):
                qk_t = qk_sb[:, t * QTILE:(t + 1) * QTILE]
                exp_t = exp_pool.tile([QTILE, N], BF16, tag="exp")
                parts = small_pool.tile([QTILE, 2], F32, tag="parts")
                s_t = small_pool.tile([QTILE, 1], F32, tag="S")
                rec_t = small_pool.tile([QTILE, 1], F32, tag="rec")
                o = out_pool.tile([QTILE, N], BF16, tag="o")
                for h in range(2):
                    ps = psum_pool.tile([QTILE, MHALF], F32, tag="ps")
                    for c in range(4):
                        m0 = h * MHALF + c * MCHUNK
                        nc.tensor.matmul(ps[:, c * MCHUNK:(c + 1) * MCHUNK],
                                         mhalf[:], mksq[:, m0:m0 + MCHUNK],
                                         start=True, stop=False)
                    for c in range(4):
                        m0 = h * MHALF + c * MCHUNK
                        nc.tensor.matmul(ps[:, c * MCHUNK:(c + 1) * MCHUNK],
                                         qk_t, mk_sb[:, m0:m0 + MCHUNK],
                                         start=False, stop=True)
                    nc.scalar.activation(
                        exp_t[:, h * MHALF:(h + 1) * MHALF], ps[:],
                        mybir.ActivationFunctionType.Exp, scale=SCALE,
                        accum_out=parts[:, h:h + 1])
                # Denominator (partials from the 2 activations) + normalize.
                nc.vector.tensor_add(s_t[:], parts[:, 0:1], parts[:, 1:2])
                nc.vector.reciprocal(rec_t[:], s_t[:])
                # Normalize + store per half: output DMA of half h starts
                # while half h+1 is still being scaled (shorter tail).
                for h in range(2):
                    nc.vector.tensor_scalar_mul(
                        o[:, h * MHALF:(h + 1) * MHALF],
                        exp_t[:, h * MHALF:(h + 1) * MHALF], rec_t[:])
                    nc.sync.dma_start(
                        out=out_d[t * QTILE:(t + 1) * QTILE,
                                  h * MHALF:(h + 1) * MHALF],
                        in_=o[:, h * MHALF:(h + 1) * MHALF])
    _strip_self_waits(nc)
    return nc


def _strip_self_waits(nc):
    """Walrus rejects instructions carrying more than one sync wait.

    Conservative fix: for any instruction with N>1 waits, keep the last wait
    on the instruction and spill the other N-1 onto single-wait Drain
    instructions inserted immediately before it on the same engine. All waits
    still execute, in program order, on the same sequencer; semaphores are
    monotonic so splitting an AND of waits into a sequence is equivalent.
    """
    for fn in nc.m.functions:
        for blk in fn.blocks:
            il = blk.instructions
            new_il = []
            changed = False
            for ins in il:
                si = getattr(ins, "sync_info", None)
                if si is not None and len(si.on_wait) > 1:
                    changed = True
                    waits = list(si.on_wait)
                    for k, w in enumerate(waits[:-1]):
                        d = mybir.InstDrain(
                            name=f"{ins.name}_w{k}",
                            ins=[], outs=[], bass_is_fusable=False)
                        d.engine = ins.engine
                        d.sync_info = bass_rust.SyncInfo(on_wait=[w],
                                                         on_update=[])
                        new_il.append(d)
                    ins.sync_info = bass_rust.SyncInfo(on_wait=[waits[-1]],
                                                      on_update=si.on_update)
                new_il.append(ins)
            if changed:
                blk.instructions = new_il


_NC_CACHE = None


def _make_in_maps(Mk: np.ndarray, Qk: np.ndarray) -> list[dict]:
    Mk = np.ascontiguousarray(np.asarray(Mk), dtype=np.float32)
    Qk = np.ascontiguousarray(np.asarray(Qk), dtype=np.float32)
    in_maps = []
    for c in range(N_CORES):
        b, half = c // 2, c % 2
        mk = np.ascontiguousarray(Mk[b].reshape(CK, N).astype(np.float16))
        qk = np.ascontiguousarray(
            Qk[b].reshape(CK, N)[:, half * QH:(half + 1) * QH]
            .astype(np.float16))
        in_maps.append({"mk": mk, "qk": qk})
    return in_maps


def kernel(Mk: np.ndarray, Qk: np.ndarray) -> np.ndarray:
    global _NC_CACHE
    if _NC_CACHE is None:
        _NC_CACHE = _build()
    nc = _NC_CACHE

    in_maps = _make_in_maps(Mk, Qk)

    res = run_bass_kernel_spmd(nc, in_maps, core_ids=list(range(N_CORES)))

    out = np.empty((B, N, N), dtype=np.float32)
    for c in range(N_CORES):
        b, half = c // 2, c % 2
        out[b, :, half * QH:(half + 1) * QH] = \
            res.results[c]["out"].astype(np.float32).T
    return out
